# revision 22
# baseline (speedup 1.0000x reference)
"""MixProp GNN message passing on 8 Trainium2 NeuronCores.

Reference computation (per batch element b):
    h0 = x;  h_k = alpha*x + (1-alpha) * (adj @ h_{k-1})   k=1..3
    ho = concat([h0..h3], channel axis);  out = W @ ho + b  (1x1 conv)

Node propagation commutes with channel mixing, so alpha-blending folds
into the conv weights on the host: out = sum_k M_k @ (A^k x) + b.
Device computes y_k = c_k A^k x (power-of-2 chain scales folded into the
psum evacuations) and the channel mix; host applies bias + output scale.

Per-core dataflow (data-parallel over batch, one element per core):
  X [128 v-part, 4 wt, (tg c dt)] fp16  <- host pre-swizzled; free dim is
  42 t-groups x 32 ch x 4 t, so a [128,128] column block is (c,dt)-pure.
  y1 = A X /2 ; y2 = A y1 /64 ; y3 = A y2     (PE fp16, DVE evacuation)
  Each y_k[:, vt, :] is transposed on-chip by the XBAR DMA-transpose unit
  (14ns per 16x128 tile) into [(c,dt), tg, v] SBUF tiles; y1/y2 are then
  downcast to fp8 by the Pool engine.
  Conv per (vt, group of 4 tg): one fp8 DoubleRow matmul (S1,S2)x(y1,y2)
  at 0.5 cyc/col + one fp8 matmul S0 x x8 + one fp16 matmul S3 x y3,
  with block-diagonal stationaries S_k[(c,dt),(o,dt')] = d(dt,dt') M'_k.
  ACT evacuates conv psum to fp16; output written as [vt,(o,dt),tg,v],
  un-permuted on the host.
"""

import sys

import numpy as np

sys.path.insert(0, "/opt/trn_rl_repo")

from contextlib import ExitStack

GDEP = 3
ALPHA = 0.05
C = 32            # channels
N = 512           # nodes
T = 168           # time steps
B = 8             # batch == n_cores
P = 128           # partitions
NVT = N // P      # 4 node tiles
DT = 4            # t-group inner size
TG = T // DT      # 42 t-groups
CT = C * T        # 5376 free columns in propagation layout

# chain scales c_k applied at each step's psum evacuation
EV1, EV2, EV3 = 0.5, 1.0 / 64.0, 1.0
C1, C2, C3 = 0.5, 1.0 / 128.0, 1.0 / 128.0
OUT_SCALE = 64.0  # device output is out/OUT_SCALE (fp16 range)

# propagation psum chunking: three groups, wt-outer accumulation inside a
# group so step 1 starts after only X[wt=0] has landed
PROP_CHUNKS = [(i * 512, 512) for i in range(10)] + [(5120, 256)]
PROP_GROUPS = [PROP_CHUNKS[:4], PROP_CHUNKS[4:8], PROP_CHUNKS[8:]]
# conv: 42 t-groups in groups of 4 (one [128,512] psum tile each)
CONV_GROUPS = [(m * 4, min(4, TG - 4 * m)) for m in range((TG + 3) // 4)]

_NC_CACHE = {}


def _build_nc():
    import concourse.mybir as mybir
    import concourse.tile as tile
    from concourse import bacc

    f32 = mybir.dt.float32
    f16 = mybir.dt.float16
    f8 = mybir.dt.float8e4

    nc = bacc.Bacc("TRN2", target_bir_lowering=False, debug=False, num_devices=B)

    xprop = nc.dram_tensor("xprop", [P, NVT, CT], f16, kind="ExternalInput").ap()
    xc8 = nc.dram_tensor("xc8", [NVT, P, TG, P], f8, kind="ExternalInput").ap()
    adjT16 = nc.dram_tensor("adjT16", [N, N], f16, kind="ExternalInput").ap()
    s12 = nc.dram_tensor("s12", [P, 2, P], f8, kind="ExternalInput").ap()
    s0 = nc.dram_tensor("s0", [P, P], f8, kind="ExternalInput").ap()
    s3 = nc.dram_tensor("s3", [P, P], f16, kind="ExternalInput").ap()
    out16 = nc.dram_tensor("out16", [NVT, P, TG, P], f16, kind="ExternalOutput").ap()

    with tile.TileContext(nc) as tc, ExitStack() as ctx:
        _emit(ctx, tc, nc, mybir, xprop, xc8, adjT16, s12, s0, s3, out16)

    nc.compile()
    return nc


def _emit(ctx, tc, nc, mybir, xprop, xc8, adjT16, s12, s0, s3, out16):
    f32 = mybir.dt.float32
    f16 = mybir.dt.float16
    f8 = mybir.dt.float8e4
    DR = mybir.MatmulPerfMode.DoubleRow

    const_pool = ctx.enter_context(tc.tile_pool(name="const", bufs=1))
    chain_pool = ctx.enter_context(tc.tile_pool(name="chain", bufs=2))
    xbar_pool = ctx.enter_context(tc.tile_pool(name="xbar", bufs=3))
    ho8_pool = ctx.enter_context(tc.tile_pool(name="ho8", bufs=4))
    xc8_pool = ctx.enter_context(tc.tile_pool(name="xc8", bufs=2))
    psum_pool = ctx.enter_context(tc.tile_pool(name="psum", bufs=5, space="PSUM"))
    cpsum_pool = ctx.enter_context(tc.tile_pool(name="cpsum", bufs=3, space="PSUM"))
    ostage_pool = ctx.enter_context(tc.tile_pool(name="ostage", bufs=12))

    # ---- adjacency + X interleaved per wt (earliest PE dependencies) ----
    adj_sb = const_pool.tile([P, NVT, N], f16, tag="adj")
    adjT_v = adjT16.rearrange("(wt wp) v -> wp wt v", wp=P)
    X = chain_pool.tile([P, NVT, CT], f16, tag="chain")
    for wt in range(NVT):
        nc.sync.dma_start(adj_sb[:, wt, :], adjT_v[:, wt, :])
        nc.sync.dma_start(X[:, wt, :], xprop[:, wt, :])

    # conv stationaries (needed ~100us later; small)
    s12_sb = const_pool.tile([P, 2, P], f8, tag="s12")
    nc.sync.dma_start(s12_sb[:], s12)
    s0_sb = const_pool.tile([P, P], f8, tag="s0")
    nc.sync.dma_start(s0_sb[:], s0)
    s3_sb = const_pool.tile([P, P], f16, tag="s3")
    nc.sync.dma_start(s3_sb[:], s3)

    ho8_tiles = {}
    y3t_tiles = {}
    xc8_tiles = {}

    def emit_step(cur, dst, vt, scale):
        """dst[:, vt, cols] = scale * sum_wt adj^T_block @ cur[:, wt, cols]."""
        for group in PROP_GROUPS:
            tiles = [
                psum_pool.tile([P, 512], f32, tag="ps", name=f"ps{j0}")
                for (j0, jn) in group
            ]
            for wt in range(NVT):
                for ps, (j0, jn) in zip(tiles, group):
                    nc.tensor.matmul(
                        ps[:, :jn],
                        adj_sb[:, wt, vt * P:(vt + 1) * P],
                        cur[:, wt, j0:j0 + jn],
                        start=(wt == 0),
                        stop=(wt == NVT - 1),
                    )
            for ps, (j0, jn) in zip(tiles, group):
                if scale == 1.0:
                    nc.vector.tensor_copy(dst[:, vt, j0:j0 + jn], ps[:, :jn])
                else:
                    nc.vector.tensor_scalar_mul(
                        dst[:, vt, j0:j0 + jn], ps[:, :jn], scale
                    )

    # transposes split into quarters: a conv can start on the first quarter
    # while later ones stream, and out-DMAs interleave on the DMA device
    XBAR_SPLITS = [(0, 11), (11, 11), (22, 10), (32, 10)]

    def emit_xbar(src, vt, eng=None):
        """Transpose src[:, vt, :] into a [(c,dt), tg, v] fp16 tile.
        y1/y2 run on SP's hwdge queue; y3 on ACT's (emitted after the
        previous conv's evacuations so it never blocks them in SEQ order)."""
        eng = eng or nc.sync
        xb = xbar_pool.tile([P, TG, P], f16, tag="xb")
        for t0, tn in XBAR_SPLITS:
            eng.dma_start_transpose(
                xb[:, t0:t0 + tn, :],
                src[:, vt, t0 * P:(t0 + tn) * P],
            )
        return xb

    def emit_xbar_fp8(src, vt, slot):
        xb = emit_xbar(src, vt)
        if vt not in ho8_tiles:
            ho8_tiles[vt] = ho8_pool.tile(
                [P, 2, TG, P], f8, tag="ho8", name=f"ho8_{vt}"
            )
        nc.gpsimd.tensor_copy(ho8_tiles[vt][:, slot, :, :], xb[:])

    def load_xc8(vt):
        xt = xc8_pool.tile([P, TG, P], f8, tag="xc8")
        nc.sync.dma_start(xt[:], xc8[vt])
        xc8_tiles[vt] = xt

    def emit_conv(vt):
        ho8 = ho8_tiles[vt]
        y3t = y3t_tiles[vt]
        xt = xc8_tiles[vt]
        for tg0, gn in CONV_GROUPS:
            w = gn * P
            cps = cpsum_pool.tile([P, 512], f32, tag="cps")
            nc.tensor.matmul(
                cps[:, :w],
                s12_sb[:],
                ho8[:, :, tg0:tg0 + gn, :],
                start=True,
                stop=False,
                perf_mode=DR,
            )
            nc.tensor.matmul(
                cps[:, :w],
                s0_sb[:],
                xt[:, tg0:tg0 + gn, :],
                start=False,
                stop=False,
            )
            nc.tensor.matmul(
                cps[:, :w],
                s3_sb[:],
                y3t[:, tg0:tg0 + gn, :],
                start=False,
                stop=True,
            )
            ot = ostage_pool.tile([P, 512], f16, tag="ot")
            # alternate the psum drain between ACT and DVE (Pool cannot read
            # PSUM) so the conv tail is PE-paced, not evacuation-paced
            if (tg0 // 4) % 2 == 0:
                nc.scalar.activation(
                    ot[:, :w], cps[:, :w], mybir.ActivationFunctionType.Identity
                )
            else:
                nc.vector.tensor_copy(ot[:, :w], cps[:, :w])
            nc.sync.dma_start(
                out16[vt, :, tg0:tg0 + gn, :],
                ot[:, :w].rearrange("p (g v) -> p g v", v=P),
            )

    # ---- step 1: X -> y1 (scale 1/2), transpose+fp8 per vt ----
    y1 = chain_pool.tile([P, NVT, CT], f16, tag="chain")
    for vt in range(NVT):
        emit_step(X, y1, vt, EV1)
        emit_xbar_fp8(y1, vt, 0)

    # ---- step 2: y1 -> y2 (scale 1/64 -> chain holds y2/128) ----
    y2 = chain_pool.tile([P, NVT, CT], f16, tag="chain")
    for vt in range(NVT):
        emit_step(y1, y2, vt, EV2)
        emit_xbar_fp8(y2, vt, 1)

    # ---- step 3 + conv, conv lagged one node tile; each y3 transpose is
    # emitted (on ACT's queue) right after the conv whose evacuations would
    # otherwise queue behind it ----
    y3s = chain_pool.tile([P, NVT, CT], f16, tag="chain")
    load_xc8(0)
    emit_step(y2, y3s, 0, EV3)
    y3t_tiles[0] = emit_xbar(y3s, 0, nc.scalar)
    load_xc8(1)
    emit_step(y2, y3s, 1, EV3)
    emit_conv(0)
    y3t_tiles[1] = emit_xbar(y3s, 1, nc.scalar)
    load_xc8(2)
    emit_step(y2, y3s, 2, EV3)
    emit_conv(1)
    y3t_tiles[2] = emit_xbar(y3s, 2, nc.scalar)
    load_xc8(3)
    emit_step(y2, y3s, 3, EV3)
    emit_conv(2)
    y3t_tiles[3] = emit_xbar(y3s, 3, nc.scalar)
    emit_conv(3)


def _host_prep(adj, W, b):
    """Constant folding: transposed adj, block-diagonal mixed conv weights."""
    import ml_dtypes

    a, be = ALPHA, 1.0 - ALPHA
    W = np.asarray(W, dtype=np.float32)
    W0, W1, W2, W3 = (W[:, i * C:(i + 1) * C] for i in range(4))
    M0 = W0 + a * (W1 + W2 + W3)
    M1 = be * (W1 + a * W2 + a * W3)
    M2 = be * be * (W2 + a * W3)
    M3 = be * be * be * W3

    def blockdiag(M):  # [o, c] -> [(c,dt), (o,dt)] with dt-diagonal
        Z = np.zeros((C, DT, C, DT), dtype=np.float32)
        for dt in range(DT):
            Z[:, dt, :, dt] = M.T
        return Z.reshape(P, P)

    s0 = blockdiag(M0 / OUT_SCALE).astype(ml_dtypes.float8_e4m3)
    s1 = blockdiag(M1 / (C1 * OUT_SCALE))
    s2 = blockdiag(M2 / (C2 * OUT_SCALE))
    s12 = np.ascontiguousarray(
        np.stack([s1, s2], axis=1)
    ).astype(ml_dtypes.float8_e4m3)  # [128, 2, 128]
    s3 = blockdiag(M3 / (C3 * OUT_SCALE)).astype(np.float16)
    adjT16 = np.ascontiguousarray(np.asarray(adj, dtype=np.float32).T).astype(
        np.float16
    )
    return adjT16, s12, s0, s3


def make_in_maps(x, adj, W, b):
    import ml_dtypes

    adjT16, s12, s0, s3 = _host_prep(adj, W, b)
    x16 = np.asarray(x, dtype=np.float32).astype(np.float16)
    # [B, c, wt, vp, tg, dt]
    xr = x16.reshape(B, C, NVT, P, TG, DT)
    xprop = np.ascontiguousarray(xr.transpose(0, 3, 2, 4, 1, 5)).reshape(
        B, P, NVT, CT
    )
    xc8 = np.ascontiguousarray(xr.transpose(0, 2, 1, 5, 4, 3)).reshape(
        B, NVT, P, TG, P
    ).astype(ml_dtypes.float8_e4m3)
    return [
        {
            "xprop": xprop[i],
            "xc8": xc8[i],
            "adjT16": adjT16,
            "s12": s12,
            "s0": s0,
            "s3": s3,
        }
        for i in range(B)
    ]


def _finish_host(raw, b):
    """raw: [B, NVT, P(odt), TG, P(vp)] fp16 -> [B, C, N, T] fp32."""
    o = np.asarray(raw).astype(np.float32).reshape(B, NVT, C, DT, TG, P)
    o = o.transpose(0, 2, 1, 5, 4, 3).reshape(B, C, N, T)
    o *= OUT_SCALE
    o += np.asarray(b, dtype=np.float32)[None, :, None, None]
    return o


def _get_nc():
    if "nc" not in _NC_CACHE:
        _NC_CACHE["nc"] = _build_nc()
    return _NC_CACHE["nc"]


def _get_runner():
    """Reusable jitted SPMD executor (safe to invoke repeatedly, unlike
    per-call run_bass_kernel_spmd under axon)."""
    if "runner" in _NC_CACHE:
        return _NC_CACHE["runner"]
    import jax
    from jax.sharding import Mesh, PartitionSpec
    try:
        from jax import shard_map
    except ImportError:
        from jax.experimental.shard_map import shard_map
    from concourse import bass2jax, mybir

    nc = _get_nc()
    bass2jax.install_neuronx_cc_hook()

    pname = nc.partition_id_tensor.name if nc.partition_id_tensor else None
    in_names, out_names, out_avals, zero_outs = [], [], [], []
    for alloc in nc.m.functions[0].allocations:
        if not isinstance(alloc, mybir.MemoryLocationSet):
            continue
        name = alloc.memorylocations[0].name
        if alloc.kind == "ExternalInput":
            if name != pname:
                in_names.append(name)
        elif alloc.kind == "ExternalOutput":
            out_names.append(name)
            shape = tuple(alloc.tensor_shape)
            dtype = mybir.dt.np(alloc.dtype)
            out_avals.append(jax.core.ShapedArray(shape, dtype))
            zero_outs.append(np.zeros(shape, dtype))
    n_params = len(in_names)
    in_names_all = list(in_names) + out_names
    if pname is not None:
        in_names_all.append(pname)

    def _body(*args):
        operands = list(args)
        if pname is not None:
            operands.append(bass2jax.partition_id_tensor())
        return tuple(
            bass2jax._bass_exec_p.bind(
                *operands,
                out_avals=tuple(out_avals),
                in_names=tuple(in_names_all),
                out_names=tuple(out_names),
                lowering_input_output_aliases=(),
                sim_require_finite=True,
                sim_require_nnan=True,
                nc=nc,
            )
        )

    devices = jax.devices()[:B]
    mesh = Mesh(np.asarray(devices), ("core",))
    sm_kwargs = dict(
        mesh=mesh,
        in_specs=(PartitionSpec("core"),) * (n_params + len(out_names)),
        out_specs=(PartitionSpec("core"),) * len(out_names),
    )
    try:
        wrapped = shard_map(_body, check_rep=False, **sm_kwargs)
    except TypeError:
        wrapped = shard_map(_body, check_vma=False, **sm_kwargs)
    fn = jax.jit(wrapped, keep_unused=True)

    def run(in_maps):
        per_core = [[np.asarray(m[nm]) for nm in in_names] for m in in_maps]
        concat_in = [
            np.concatenate([per_core[c][i] for c in range(B)], axis=0)
            for i in range(n_params)
        ]
        concat_zero = [np.concatenate([z] * B, axis=0) for z in zero_outs]
        outs = fn(*concat_in, *concat_zero)
        oi = out_names.index("out16")
        full = np.asarray(outs[oi])
        per_core_rows = out_avals[oi].shape[0]
        return full.reshape(B, per_core_rows, *out_avals[oi].shape[1:])

    _NC_CACHE["runner"] = run
    return run


def kernel(x, adj, W, b):
    in_maps = make_in_maps(x, adj, W, b)
    try:
        run = _get_runner()
        raw = run(in_maps)
    except Exception:
        from concourse.bass_utils import run_bass_kernel_spmd

        res = run_bass_kernel_spmd(_get_nc(), in_maps, list(range(B)))
        raw = np.stack([res.results[i]["out16"] for i in range(B)], axis=0)
    return _finish_host(raw, b)


# revision 25
# speedup vs baseline: 1.0866x; 1.0866x over previous
"""MixProp GNN message passing on 8 Trainium2 NeuronCores.

Reference computation (per batch element b):
    h0 = x;  h_k = alpha*x + (1-alpha) * (adj @ h_{k-1})   k=1..3
    ho = concat([h0..h3], channel axis);  out = W @ ho + b  (1x1 conv)

Node propagation commutes with channel mixing, so alpha-blending folds
into the conv weights on the host: out = sum_k M_k @ (A^k x) + b.
Device computes y_k = c_k A^k x (power-of-2 chain scales folded into the
psum evacuations) and the channel mix; host applies bias + output scale.

Per-core dataflow (data-parallel over batch, one element per core):
  X [128 v-part, 4 wt, (tg c dt)] fp16  <- host pre-swizzled; free dim is
  42 t-groups x 32 ch x 4 t, so a [128,128] column block is (c,dt)-pure.
  y1 = A X /2 ; y2 = A y1 /64 ; y3 = A y2     (PE fp16, DVE evacuation)
  Each y_k[:, vt, :] is transposed on-chip by the XBAR DMA-transpose unit
  (14ns per 16x128 tile) into [(c,dt), tg, v] SBUF tiles; y1/y2 are then
  downcast to fp8 by the Pool engine.
  Conv per (vt, group of 4 tg): one fp8 DoubleRow matmul (S1,S2)x(y1,y2)
  at 0.5 cyc/col + one fp8 matmul S0 x x8 + one fp16 matmul S3 x y3,
  with block-diagonal stationaries S_k[(c,dt),(o,dt')] = d(dt,dt') M'_k.
  ACT evacuates conv psum to fp16; output written as [vt,(o,dt),tg,v],
  un-permuted on the host.
"""

import sys

import numpy as np

sys.path.insert(0, "/opt/trn_rl_repo")

from contextlib import ExitStack

GDEP = 3
ALPHA = 0.05
C = 32            # channels
N = 512           # nodes
T = 168           # time steps
B = 8             # batch == n_cores
P = 128           # partitions
NVT = N // P      # 4 node tiles
DT = 4            # t-group inner size
TG = T // DT      # 42 t-groups
CT = C * T        # 5376 free columns in propagation layout

# chain scales c_k applied at each step's psum evacuation
EV1, EV2, EV3 = 0.5, 1.0 / 64.0, 1.0
C1, C2, C3 = 0.5, 1.0 / 128.0, 1.0 / 128.0
OUT_SCALE = 64.0  # device output is out/OUT_SCALE (fp16 range)

# propagation psum chunking: three groups, wt-outer accumulation inside a
# group so step 1 starts after only X[wt=0] has landed
PROP_CHUNKS = [(i * 512, 512) for i in range(10)] + [(5120, 256)]
PROP_GROUPS = [PROP_CHUNKS[:4], PROP_CHUNKS[4:8], PROP_CHUNKS[8:]]
# conv: 42 t-groups in groups of 4 (one [128,512] psum tile each)
CONV_GROUPS = [(m * 4, min(4, TG - 4 * m)) for m in range((TG + 3) // 4)]

_NC_CACHE = {}


def _build_nc():
    import concourse.mybir as mybir
    import concourse.tile as tile
    from concourse import bacc

    f32 = mybir.dt.float32
    f16 = mybir.dt.float16
    f8 = mybir.dt.float8e4

    nc = bacc.Bacc("TRN2", target_bir_lowering=False, debug=False, num_devices=B)

    xprop = nc.dram_tensor("xprop", [P, NVT, CT], f16, kind="ExternalInput").ap()
    xc8 = nc.dram_tensor("xc8", [NVT, P, TG, P], f8, kind="ExternalInput").ap()
    adjT16 = nc.dram_tensor("adjT16", [N, N], f16, kind="ExternalInput").ap()
    s12 = nc.dram_tensor("s12", [P, 2, P], f8, kind="ExternalInput").ap()
    s0 = nc.dram_tensor("s0", [P, P], f8, kind="ExternalInput").ap()
    s3 = nc.dram_tensor("s3", [P, P], f16, kind="ExternalInput").ap()
    out16 = nc.dram_tensor("out16", [NVT, P, TG, P], f16, kind="ExternalOutput").ap()

    with tile.TileContext(nc) as tc, ExitStack() as ctx:
        _emit(ctx, tc, nc, mybir, xprop, xc8, adjT16, s12, s0, s3, out16)

    nc.compile()
    return nc


def _emit(ctx, tc, nc, mybir, xprop, xc8, adjT16, s12, s0, s3, out16):
    f32 = mybir.dt.float32
    f16 = mybir.dt.float16
    f8 = mybir.dt.float8e4
    DR = mybir.MatmulPerfMode.DoubleRow

    const_pool = ctx.enter_context(tc.tile_pool(name="const", bufs=1))
    chain_pool = ctx.enter_context(tc.tile_pool(name="chain", bufs=2))
    xbar_pool = ctx.enter_context(tc.tile_pool(name="xbar", bufs=3))
    ho8_pool = ctx.enter_context(tc.tile_pool(name="ho8", bufs=4))
    xc8_pool = ctx.enter_context(tc.tile_pool(name="xc8", bufs=2))
    psum_pool = ctx.enter_context(tc.tile_pool(name="psum", bufs=5, space="PSUM"))
    cpsum_pool = ctx.enter_context(tc.tile_pool(name="cpsum", bufs=3, space="PSUM"))
    ostage_pool = ctx.enter_context(tc.tile_pool(name="ostage", bufs=12))

    # ---- adjacency + X interleaved per wt (earliest PE dependencies) ----
    adj_sb = const_pool.tile([P, NVT, N], f16, tag="adj")
    adjT_v = adjT16.rearrange("(wt wp) v -> wp wt v", wp=P)
    X = chain_pool.tile([P, NVT, CT], f16, tag="chain")
    for wt in range(NVT):
        nc.sync.dma_start(adj_sb[:, wt, :], adjT_v[:, wt, :])
        nc.sync.dma_start(X[:, wt, :], xprop[:, wt, :])

    # conv stationaries (needed ~100us later; small)
    s12_sb = const_pool.tile([P, 2, P], f8, tag="s12")
    nc.sync.dma_start(s12_sb[:], s12)
    s0_sb = const_pool.tile([P, P], f8, tag="s0")
    nc.sync.dma_start(s0_sb[:], s0)
    s3_sb = const_pool.tile([P, P], f16, tag="s3")
    nc.sync.dma_start(s3_sb[:], s3)

    ho8_tiles = {}
    y3t_tiles = {}
    xc8_tiles = {}

    def emit_step(cur, dst, vt, scale):
        """dst[:, vt, cols] = scale * sum_wt adj^T_block @ cur[:, wt, cols]."""
        for group in PROP_GROUPS:
            tiles = [
                psum_pool.tile([P, 512], f32, tag="ps", name=f"ps{j0}")
                for (j0, jn) in group
            ]
            for wt in range(NVT):
                for ps, (j0, jn) in zip(tiles, group):
                    nc.tensor.matmul(
                        ps[:, :jn],
                        adj_sb[:, wt, vt * P:(vt + 1) * P],
                        cur[:, wt, j0:j0 + jn],
                        start=(wt == 0),
                        stop=(wt == NVT - 1),
                    )
            for ps, (j0, jn) in zip(tiles, group):
                if scale == 1.0:
                    nc.vector.tensor_copy(dst[:, vt, j0:j0 + jn], ps[:, :jn])
                else:
                    nc.vector.tensor_scalar_mul(
                        dst[:, vt, j0:j0 + jn], ps[:, :jn], scale
                    )

    # transposes split into quarters: a conv can start on the first quarter
    # while later ones stream, and out-DMAs interleave on the DMA device
    XBAR_SPLITS = [(0, 11), (11, 11), (22, 10), (32, 10)]

    def emit_xbar(src, vt, eng=None):
        """Transpose src[:, vt, :] into a [(c,dt), tg, v] fp16 tile. All
        transposes run on SP's hwdge queue, which carries nothing else that
        is latency-critical (out DMAs live on ACT's queue)."""
        eng = eng or nc.sync
        xb = xbar_pool.tile([P, TG, P], f16, tag="xb")
        for t0, tn in XBAR_SPLITS:
            eng.dma_start_transpose(
                xb[:, t0:t0 + tn, :],
                src[:, vt, t0 * P:(t0 + tn) * P],
            )
        return xb

    def emit_xbar_fp8(src, vt, slot):
        xb = emit_xbar(src, vt)
        if vt not in ho8_tiles:
            ho8_tiles[vt] = ho8_pool.tile(
                [P, 2, TG, P], f8, tag="ho8", name=f"ho8_{vt}"
            )
        nc.gpsimd.tensor_copy(ho8_tiles[vt][:, slot, :, :], xb[:])

    def load_xc8(vt):
        xt = xc8_pool.tile([P, TG, P], f8, tag="xc8")
        nc.sync.dma_start(xt[:], xc8[vt])
        xc8_tiles[vt] = xt

    def emit_conv(vt):
        ho8 = ho8_tiles[vt]
        y3t = y3t_tiles[vt]
        xt = xc8_tiles[vt]
        for tg0, gn in CONV_GROUPS:
            w = gn * P
            cps = cpsum_pool.tile([P, 512], f32, tag="cps")
            nc.tensor.matmul(
                cps[:, :w],
                s12_sb[:],
                ho8[:, :, tg0:tg0 + gn, :],
                start=True,
                stop=False,
                perf_mode=DR,
            )
            nc.tensor.matmul(
                cps[:, :w],
                s0_sb[:],
                xt[:, tg0:tg0 + gn, :],
                start=False,
                stop=False,
            )
            nc.tensor.matmul(
                cps[:, :w],
                s3_sb[:],
                y3t[:, tg0:tg0 + gn, :],
                start=False,
                stop=True,
            )
            ot = ostage_pool.tile([P, 512], f16, tag="ot")
            # DVE drains conv psum (Pool cannot read PSUM); the out DMA is
            # issued from ACT's queue so transposes on SP are never stuck
            # behind an out DMA whose data is not ready yet
            nc.vector.tensor_copy(ot[:, :w], cps[:, :w])
            nc.scalar.dma_start(
                out16[vt, :, tg0:tg0 + gn, :],
                ot[:, :w].rearrange("p (g v) -> p g v", v=P),
            )

    # ---- step 1: X -> y1 (scale 1/2), transpose+fp8 per vt ----
    y1 = chain_pool.tile([P, NVT, CT], f16, tag="chain")
    for vt in range(NVT):
        emit_step(X, y1, vt, EV1)
        emit_xbar_fp8(y1, vt, 0)

    # ---- step 2: y1 -> y2 (scale 1/64 -> chain holds y2/128) ----
    y2 = chain_pool.tile([P, NVT, CT], f16, tag="chain")
    for vt in range(NVT):
        emit_step(y1, y2, vt, EV2)
        emit_xbar_fp8(y2, vt, 1)

    # ---- step 3 + conv, conv lagged one node tile; each y3 transpose is
    # emitted (on ACT's queue) right after the conv whose evacuations would
    # otherwise queue behind it ----
    y3s = chain_pool.tile([P, NVT, CT], f16, tag="chain")
    load_xc8(0)
    emit_step(y2, y3s, 0, EV3)
    y3t_tiles[0] = emit_xbar(y3s, 0)
    load_xc8(1)
    emit_step(y2, y3s, 1, EV3)
    emit_conv(0)
    y3t_tiles[1] = emit_xbar(y3s, 1)
    load_xc8(2)
    emit_step(y2, y3s, 2, EV3)
    emit_conv(1)
    y3t_tiles[2] = emit_xbar(y3s, 2)
    load_xc8(3)
    emit_step(y2, y3s, 3, EV3)
    emit_conv(2)
    y3t_tiles[3] = emit_xbar(y3s, 3)
    emit_conv(3)


def _host_prep(adj, W, b):
    """Constant folding: transposed adj, block-diagonal mixed conv weights."""
    import ml_dtypes

    a, be = ALPHA, 1.0 - ALPHA
    W = np.asarray(W, dtype=np.float32)
    W0, W1, W2, W3 = (W[:, i * C:(i + 1) * C] for i in range(4))
    M0 = W0 + a * (W1 + W2 + W3)
    M1 = be * (W1 + a * W2 + a * W3)
    M2 = be * be * (W2 + a * W3)
    M3 = be * be * be * W3

    def blockdiag(M):  # [o, c] -> [(c,dt), (o,dt)] with dt-diagonal
        Z = np.zeros((C, DT, C, DT), dtype=np.float32)
        for dt in range(DT):
            Z[:, dt, :, dt] = M.T
        return Z.reshape(P, P)

    s0 = blockdiag(M0 / OUT_SCALE).astype(ml_dtypes.float8_e4m3)
    s1 = blockdiag(M1 / (C1 * OUT_SCALE))
    s2 = blockdiag(M2 / (C2 * OUT_SCALE))
    s12 = np.ascontiguousarray(
        np.stack([s1, s2], axis=1)
    ).astype(ml_dtypes.float8_e4m3)  # [128, 2, 128]
    s3 = blockdiag(M3 / (C3 * OUT_SCALE)).astype(np.float16)
    adjT16 = np.ascontiguousarray(np.asarray(adj, dtype=np.float32).T).astype(
        np.float16
    )
    return adjT16, s12, s0, s3


def make_in_maps(x, adj, W, b):
    import ml_dtypes

    adjT16, s12, s0, s3 = _host_prep(adj, W, b)
    x16 = np.asarray(x, dtype=np.float32).astype(np.float16)
    # [B, c, wt, vp, tg, dt]
    xr = x16.reshape(B, C, NVT, P, TG, DT)
    xprop = np.ascontiguousarray(xr.transpose(0, 3, 2, 4, 1, 5)).reshape(
        B, P, NVT, CT
    )
    xc8 = np.ascontiguousarray(xr.transpose(0, 2, 1, 5, 4, 3)).reshape(
        B, NVT, P, TG, P
    ).astype(ml_dtypes.float8_e4m3)
    return [
        {
            "xprop": xprop[i],
            "xc8": xc8[i],
            "adjT16": adjT16,
            "s12": s12,
            "s0": s0,
            "s3": s3,
        }
        for i in range(B)
    ]


def _finish_host(raw, b):
    """raw: [B, NVT, P(odt), TG, P(vp)] fp16 -> [B, C, N, T] fp32."""
    o = np.asarray(raw).astype(np.float32).reshape(B, NVT, C, DT, TG, P)
    o = o.transpose(0, 2, 1, 5, 4, 3).reshape(B, C, N, T)
    o *= OUT_SCALE
    o += np.asarray(b, dtype=np.float32)[None, :, None, None]
    return o


def _get_nc():
    if "nc" not in _NC_CACHE:
        _NC_CACHE["nc"] = _build_nc()
    return _NC_CACHE["nc"]


def _get_runner():
    """Reusable jitted SPMD executor (safe to invoke repeatedly, unlike
    per-call run_bass_kernel_spmd under axon)."""
    if "runner" in _NC_CACHE:
        return _NC_CACHE["runner"]
    import jax
    from jax.sharding import Mesh, PartitionSpec
    try:
        from jax import shard_map
    except ImportError:
        from jax.experimental.shard_map import shard_map
    from concourse import bass2jax, mybir

    nc = _get_nc()
    bass2jax.install_neuronx_cc_hook()

    pname = nc.partition_id_tensor.name if nc.partition_id_tensor else None
    in_names, out_names, out_avals, zero_outs = [], [], [], []
    for alloc in nc.m.functions[0].allocations:
        if not isinstance(alloc, mybir.MemoryLocationSet):
            continue
        name = alloc.memorylocations[0].name
        if alloc.kind == "ExternalInput":
            if name != pname:
                in_names.append(name)
        elif alloc.kind == "ExternalOutput":
            out_names.append(name)
            shape = tuple(alloc.tensor_shape)
            dtype = mybir.dt.np(alloc.dtype)
            out_avals.append(jax.core.ShapedArray(shape, dtype))
            zero_outs.append(np.zeros(shape, dtype))
    n_params = len(in_names)
    in_names_all = list(in_names) + out_names
    if pname is not None:
        in_names_all.append(pname)

    def _body(*args):
        operands = list(args)
        if pname is not None:
            operands.append(bass2jax.partition_id_tensor())
        return tuple(
            bass2jax._bass_exec_p.bind(
                *operands,
                out_avals=tuple(out_avals),
                in_names=tuple(in_names_all),
                out_names=tuple(out_names),
                lowering_input_output_aliases=(),
                sim_require_finite=True,
                sim_require_nnan=True,
                nc=nc,
            )
        )

    devices = jax.devices()[:B]
    mesh = Mesh(np.asarray(devices), ("core",))
    sm_kwargs = dict(
        mesh=mesh,
        in_specs=(PartitionSpec("core"),) * (n_params + len(out_names)),
        out_specs=(PartitionSpec("core"),) * len(out_names),
    )
    try:
        wrapped = shard_map(_body, check_rep=False, **sm_kwargs)
    except TypeError:
        wrapped = shard_map(_body, check_vma=False, **sm_kwargs)
    fn = jax.jit(wrapped, keep_unused=True)

    def run(in_maps):
        per_core = [[np.asarray(m[nm]) for nm in in_names] for m in in_maps]
        concat_in = [
            np.concatenate([per_core[c][i] for c in range(B)], axis=0)
            for i in range(n_params)
        ]
        concat_zero = [np.concatenate([z] * B, axis=0) for z in zero_outs]
        outs = fn(*concat_in, *concat_zero)
        oi = out_names.index("out16")
        full = np.asarray(outs[oi])
        per_core_rows = out_avals[oi].shape[0]
        return full.reshape(B, per_core_rows, *out_avals[oi].shape[1:])

    _NC_CACHE["runner"] = run
    return run


def kernel(x, adj, W, b):
    in_maps = make_in_maps(x, adj, W, b)
    try:
        run = _get_runner()
        raw = run(in_maps)
    except Exception:
        from concourse.bass_utils import run_bass_kernel_spmd

        res = run_bass_kernel_spmd(_get_nc(), in_maps, list(range(B)))
        raw = np.stack([res.results[i]["out16"] for i in range(B)], axis=0)
    return _finish_host(raw, b)


# revision 28
# speedup vs baseline: 1.1391x; 1.0484x over previous
"""MixProp GNN message passing on 8 Trainium2 NeuronCores.

Reference computation (per batch element b):
    h0 = x;  h_k = alpha*x + (1-alpha) * (adj @ h_{k-1})   k=1..3
    ho = concat([h0..h3], channel axis);  out = W @ ho + b  (1x1 conv)

Node propagation commutes with channel mixing, so alpha-blending folds
into the conv weights on the host: out = sum_k M_k @ (A^k x) + b.
Device computes y_k = c_k A^k x (power-of-2 chain scales folded into the
psum evacuations) and the channel mix; host applies bias + output scale.

Per-core dataflow (data-parallel over batch, one element per core):
  X [128 v-part, 4 wt, (tg c dt)] fp16  <- host pre-swizzled; free dim is
  42 t-groups x 32 ch x 4 t, so a [128,128] column block is (c,dt)-pure.
  y1 = A X /2 ; y2 = A y1 /64 ; y3 = A y2     (PE fp16, DVE evacuation)
  Each y_k[:, vt, :] is transposed on-chip by the XBAR DMA-transpose unit
  (14ns per 16x128 tile) into [(c,dt), tg, v] SBUF tiles; y1/y2 are then
  downcast to fp8 by the Pool engine.
  Conv per (vt, group of 4 tg): one fp8 DoubleRow matmul (S1,S2)x(y1,y2)
  at 0.5 cyc/col + one fp8 matmul S0 x x8 + one fp16 matmul S3 x y3,
  with block-diagonal stationaries S_k[(c,dt),(o,dt')] = d(dt,dt') M'_k.
  ACT evacuates conv psum to fp16; output written as [vt,(o,dt),tg,v],
  un-permuted on the host.
"""

import sys

import numpy as np

sys.path.insert(0, "/opt/trn_rl_repo")

from contextlib import ExitStack

GDEP = 3
ALPHA = 0.05
C = 32            # channels
N = 512           # nodes
T = 168           # time steps
B = 8             # batch == n_cores
P = 128           # partitions
NVT = N // P      # 4 node tiles
DT = 4            # t-group inner size
TG = T // DT      # 42 t-groups
CT = C * T        # 5376 free columns in propagation layout

# chain scales c_k applied at each step's psum evacuation
EV1, EV2, EV3 = 0.5, 1.0 / 64.0, 1.0
C1, C2, C3 = 0.5, 1.0 / 128.0, 1.0 / 128.0
OUT_SCALE = 64.0  # device output is out/OUT_SCALE (fp16 range)

# propagation psum chunking: three groups, wt-outer accumulation inside a
# group so step 1 starts after only X[wt=0] has landed
PROP_CHUNKS = [(i * 512, 512) for i in range(10)] + [(5120, 256)]
PROP_GROUPS = [PROP_CHUNKS[:4], PROP_CHUNKS[4:8], PROP_CHUNKS[8:]]
# conv: 42 t-groups in groups of 4 (one [128,512] psum tile each)
CONV_GROUPS = [(m * 4, min(4, TG - 4 * m)) for m in range((TG + 3) // 4)]

_NC_CACHE = {}


def _build_nc():
    import concourse.mybir as mybir
    import concourse.tile as tile
    from concourse import bacc

    f32 = mybir.dt.float32
    f16 = mybir.dt.float16
    f8 = mybir.dt.float8e4

    nc = bacc.Bacc("TRN2", target_bir_lowering=False, debug=False, num_devices=B)

    xprop = nc.dram_tensor("xprop", [P, NVT, CT], f16, kind="ExternalInput").ap()
    xc8 = nc.dram_tensor("xc8", [NVT, P, TG, P], f8, kind="ExternalInput").ap()
    adjT16 = nc.dram_tensor("adjT16", [N, N], f16, kind="ExternalInput").ap()
    s12 = nc.dram_tensor("s12", [P, 2, P], f8, kind="ExternalInput").ap()
    s0 = nc.dram_tensor("s0", [P, P], f8, kind="ExternalInput").ap()
    s3 = nc.dram_tensor("s3", [P, P], f16, kind="ExternalInput").ap()
    out16 = nc.dram_tensor("out16", [NVT, P, TG, P], f16, kind="ExternalOutput").ap()

    with tile.TileContext(nc) as tc, ExitStack() as ctx:
        _emit(ctx, tc, nc, mybir, xprop, xc8, adjT16, s12, s0, s3, out16)

    nc.compile()
    return nc


def _emit(ctx, tc, nc, mybir, xprop, xc8, adjT16, s12, s0, s3, out16):
    f32 = mybir.dt.float32
    f16 = mybir.dt.float16
    f8 = mybir.dt.float8e4
    DR = mybir.MatmulPerfMode.DoubleRow

    const_pool = ctx.enter_context(tc.tile_pool(name="const", bufs=1))
    chain_pool = ctx.enter_context(tc.tile_pool(name="chain", bufs=2))
    xbar_pool = ctx.enter_context(tc.tile_pool(name="xbar", bufs=3))
    ho8_pool = ctx.enter_context(tc.tile_pool(name="ho8", bufs=4))
    xc8_pool = ctx.enter_context(tc.tile_pool(name="xc8", bufs=2))
    psum_pool = ctx.enter_context(tc.tile_pool(name="psum", bufs=5, space="PSUM"))
    cpsum_pool = ctx.enter_context(tc.tile_pool(name="cpsum", bufs=3, space="PSUM"))
    ostage_pool = ctx.enter_context(tc.tile_pool(name="ostage", bufs=4))

    # ---- adjacency + X interleaved per wt (earliest PE dependencies) ----
    adj_sb = const_pool.tile([P, NVT, N], f16, tag="adj")
    adjT_v = adjT16.rearrange("(wt wp) v -> wp wt v", wp=P)
    X = chain_pool.tile([P, NVT, CT], f16, tag="chain")
    for wt in range(NVT):
        nc.sync.dma_start(adj_sb[:, wt, :], adjT_v[:, wt, :])
        nc.sync.dma_start(X[:, wt, :], xprop[:, wt, :])

    # conv stationaries (needed ~100us later; small)
    s12_sb = const_pool.tile([P, 2, P], f8, tag="s12")
    nc.sync.dma_start(s12_sb[:], s12)
    s0_sb = const_pool.tile([P, P], f8, tag="s0")
    nc.sync.dma_start(s0_sb[:], s0)
    s3_sb = const_pool.tile([P, P], f16, tag="s3")
    nc.sync.dma_start(s3_sb[:], s3)

    ho8_tiles = {}
    y3t_tiles = {}
    xc8_tiles = {}

    def emit_step(cur, dst, vt, scale):
        """dst[:, vt, cols] = scale * sum_wt adj^T_block @ cur[:, wt, cols]."""
        for group in PROP_GROUPS:
            tiles = [
                psum_pool.tile([P, 512], f32, tag="ps", name=f"ps{j0}")
                for (j0, jn) in group
            ]
            for wt in range(NVT):
                for ps, (j0, jn) in zip(tiles, group):
                    nc.tensor.matmul(
                        ps[:, :jn],
                        adj_sb[:, wt, vt * P:(vt + 1) * P],
                        cur[:, wt, j0:j0 + jn],
                        start=(wt == 0),
                        stop=(wt == NVT - 1),
                    )
            for ps, (j0, jn) in zip(tiles, group):
                if scale == 1.0:
                    nc.vector.tensor_copy(dst[:, vt, j0:j0 + jn], ps[:, :jn])
                else:
                    nc.vector.tensor_scalar_mul(
                        dst[:, vt, j0:j0 + jn], ps[:, :jn], scale
                    )

    # transposes split into quarters: a conv can start on the first quarter
    # while later ones stream, and out-DMAs interleave on the DMA device
    XBAR_SPLITS = [(0, 11), (11, 11), (22, 10), (32, 10)]

    def emit_xbar(src, vt, eng=None):
        """Transpose src[:, vt, :] into a [(c,dt), tg, v] fp16 tile. All
        transposes run on SP's hwdge queue, which carries nothing else that
        is latency-critical (out DMAs live on ACT's queue)."""
        eng = eng or nc.sync
        xb = xbar_pool.tile([P, TG, P], f16, tag="xb")
        for t0, tn in XBAR_SPLITS:
            eng.dma_start_transpose(
                xb[:, t0:t0 + tn, :],
                src[:, vt, t0 * P:(t0 + tn) * P],
            )
        return xb

    def emit_xbar_fp8(src, vt, slot):
        xb = emit_xbar(src, vt)
        if vt not in ho8_tiles:
            ho8_tiles[vt] = ho8_pool.tile(
                [P, 2, TG, P], f8, tag="ho8", name=f"ho8_{vt}"
            )
        nc.gpsimd.tensor_copy(ho8_tiles[vt][:, slot, :, :], xb[:])

    def load_xc8(vt):
        xt = xc8_pool.tile([P, TG, P], f8, tag="xc8")
        nc.sync.dma_start(xt[:], xc8[vt])
        xc8_tiles[vt] = xt

    def emit_conv(vt):
        ho8 = ho8_tiles[vt]
        y3t = y3t_tiles[vt]
        xt = xc8_tiles[vt]
        ot = None
        for gi, (tg0, gn) in enumerate(CONV_GROUPS):
            w = gn * P
            cps = cpsum_pool.tile([P, 512], f32, tag="cps")
            nc.tensor.matmul(
                cps[:, :w],
                s12_sb[:],
                ho8[:, :, tg0:tg0 + gn, :],
                start=True,
                stop=False,
                perf_mode=DR,
            )
            nc.tensor.matmul(
                cps[:, :w],
                s0_sb[:],
                xt[:, tg0:tg0 + gn, :],
                start=False,
                stop=False,
            )
            nc.tensor.matmul(
                cps[:, :w],
                s3_sb[:],
                y3t[:, tg0:tg0 + gn, :],
                start=False,
                stop=True,
            )
            # drain conv psum into a 4-group staging tile (alternating
            # ACT/DVE so the drain outpaces PE; Pool cannot read PSUM), then
            # one out DMA per 16 t-groups from ACT's queue, where it sits
            # right after the evacuation it depends on
            bi = gi % 4
            if bi == 0:
                bt0 = tg0
                ot = ostage_pool.tile([P, 4, 512], f16, tag="ot", name=f"ot{gi}")
            if gi % 2 == 0:
                nc.scalar.activation(
                    ot[:, bi, :w], cps[:, :w],
                    mybir.ActivationFunctionType.Identity,
                )
            else:
                nc.vector.tensor_copy(ot[:, bi, :w], cps[:, :w])
            if bi == 3 or (tg0, gn) == CONV_GROUPS[-1]:
                btn = tg0 + gn - bt0
                nc.scalar.dma_start(
                    out16[vt, :, bt0:bt0 + btn, :],
                    ot[:].rearrange("p b (g v) -> p (b g) v", v=P)[:, :btn, :],
                )

    # ---- step 1: X -> y1 (scale 1/2), transpose+fp8 per vt ----
    y1 = chain_pool.tile([P, NVT, CT], f16, tag="chain")
    for vt in range(NVT):
        emit_step(X, y1, vt, EV1)
        emit_xbar_fp8(y1, vt, 0)

    # ---- step 2: y1 -> y2 (scale 1/64 -> chain holds y2/128) ----
    y2 = chain_pool.tile([P, NVT, CT], f16, tag="chain")
    for vt in range(NVT):
        emit_step(y1, y2, vt, EV2)
        emit_xbar_fp8(y2, vt, 1)

    # ---- step 3 + conv, conv lagged one node tile; each y3 transpose is
    # emitted (on ACT's queue) right after the conv whose evacuations would
    # otherwise queue behind it ----
    y3s = chain_pool.tile([P, NVT, CT], f16, tag="chain")
    load_xc8(0)
    emit_step(y2, y3s, 0, EV3)
    y3t_tiles[0] = emit_xbar(y3s, 0)
    load_xc8(1)
    emit_step(y2, y3s, 1, EV3)
    emit_conv(0)
    y3t_tiles[1] = emit_xbar(y3s, 1)
    load_xc8(2)
    emit_step(y2, y3s, 2, EV3)
    emit_conv(1)
    y3t_tiles[2] = emit_xbar(y3s, 2)
    load_xc8(3)
    emit_step(y2, y3s, 3, EV3)
    emit_conv(2)
    y3t_tiles[3] = emit_xbar(y3s, 3)
    emit_conv(3)


def _host_prep(adj, W, b):
    """Constant folding: transposed adj, block-diagonal mixed conv weights."""
    import ml_dtypes

    a, be = ALPHA, 1.0 - ALPHA
    W = np.asarray(W, dtype=np.float32)
    W0, W1, W2, W3 = (W[:, i * C:(i + 1) * C] for i in range(4))
    M0 = W0 + a * (W1 + W2 + W3)
    M1 = be * (W1 + a * W2 + a * W3)
    M2 = be * be * (W2 + a * W3)
    M3 = be * be * be * W3

    def blockdiag(M):  # [o, c] -> [(c,dt), (o,dt)] with dt-diagonal
        Z = np.zeros((C, DT, C, DT), dtype=np.float32)
        for dt in range(DT):
            Z[:, dt, :, dt] = M.T
        return Z.reshape(P, P)

    s0 = blockdiag(M0 / OUT_SCALE).astype(ml_dtypes.float8_e4m3)
    s1 = blockdiag(M1 / (C1 * OUT_SCALE))
    s2 = blockdiag(M2 / (C2 * OUT_SCALE))
    s12 = np.ascontiguousarray(
        np.stack([s1, s2], axis=1)
    ).astype(ml_dtypes.float8_e4m3)  # [128, 2, 128]
    s3 = blockdiag(M3 / (C3 * OUT_SCALE)).astype(np.float16)
    adjT16 = np.ascontiguousarray(np.asarray(adj, dtype=np.float32).T).astype(
        np.float16
    )
    return adjT16, s12, s0, s3


def make_in_maps(x, adj, W, b):
    import ml_dtypes

    adjT16, s12, s0, s3 = _host_prep(adj, W, b)
    x16 = np.asarray(x, dtype=np.float32).astype(np.float16)
    # [B, c, wt, vp, tg, dt]
    xr = x16.reshape(B, C, NVT, P, TG, DT)
    xprop = np.ascontiguousarray(xr.transpose(0, 3, 2, 4, 1, 5)).reshape(
        B, P, NVT, CT
    )
    xc8 = np.ascontiguousarray(xr.transpose(0, 2, 1, 5, 4, 3)).reshape(
        B, NVT, P, TG, P
    ).astype(ml_dtypes.float8_e4m3)
    return [
        {
            "xprop": xprop[i],
            "xc8": xc8[i],
            "adjT16": adjT16,
            "s12": s12,
            "s0": s0,
            "s3": s3,
        }
        for i in range(B)
    ]


def _finish_host(raw, b):
    """raw: [B, NVT, P(odt), TG, P(vp)] fp16 -> [B, C, N, T] fp32."""
    o = np.asarray(raw).astype(np.float32).reshape(B, NVT, C, DT, TG, P)
    o = o.transpose(0, 2, 1, 5, 4, 3).reshape(B, C, N, T)
    o *= OUT_SCALE
    o += np.asarray(b, dtype=np.float32)[None, :, None, None]
    return o


def _get_nc():
    if "nc" not in _NC_CACHE:
        _NC_CACHE["nc"] = _build_nc()
    return _NC_CACHE["nc"]


def _get_runner():
    """Reusable jitted SPMD executor (safe to invoke repeatedly, unlike
    per-call run_bass_kernel_spmd under axon)."""
    if "runner" in _NC_CACHE:
        return _NC_CACHE["runner"]
    import jax
    from jax.sharding import Mesh, PartitionSpec
    try:
        from jax import shard_map
    except ImportError:
        from jax.experimental.shard_map import shard_map
    from concourse import bass2jax, mybir

    nc = _get_nc()
    bass2jax.install_neuronx_cc_hook()

    pname = nc.partition_id_tensor.name if nc.partition_id_tensor else None
    in_names, out_names, out_avals, zero_outs = [], [], [], []
    for alloc in nc.m.functions[0].allocations:
        if not isinstance(alloc, mybir.MemoryLocationSet):
            continue
        name = alloc.memorylocations[0].name
        if alloc.kind == "ExternalInput":
            if name != pname:
                in_names.append(name)
        elif alloc.kind == "ExternalOutput":
            out_names.append(name)
            shape = tuple(alloc.tensor_shape)
            dtype = mybir.dt.np(alloc.dtype)
            out_avals.append(jax.core.ShapedArray(shape, dtype))
            zero_outs.append(np.zeros(shape, dtype))
    n_params = len(in_names)
    in_names_all = list(in_names) + out_names
    if pname is not None:
        in_names_all.append(pname)

    def _body(*args):
        operands = list(args)
        if pname is not None:
            operands.append(bass2jax.partition_id_tensor())
        return tuple(
            bass2jax._bass_exec_p.bind(
                *operands,
                out_avals=tuple(out_avals),
                in_names=tuple(in_names_all),
                out_names=tuple(out_names),
                lowering_input_output_aliases=(),
                sim_require_finite=True,
                sim_require_nnan=True,
                nc=nc,
            )
        )

    devices = jax.devices()[:B]
    mesh = Mesh(np.asarray(devices), ("core",))
    sm_kwargs = dict(
        mesh=mesh,
        in_specs=(PartitionSpec("core"),) * (n_params + len(out_names)),
        out_specs=(PartitionSpec("core"),) * len(out_names),
    )
    try:
        wrapped = shard_map(_body, check_rep=False, **sm_kwargs)
    except TypeError:
        wrapped = shard_map(_body, check_vma=False, **sm_kwargs)
    fn = jax.jit(wrapped, keep_unused=True)

    def run(in_maps):
        per_core = [[np.asarray(m[nm]) for nm in in_names] for m in in_maps]
        concat_in = [
            np.concatenate([per_core[c][i] for c in range(B)], axis=0)
            for i in range(n_params)
        ]
        concat_zero = [np.concatenate([z] * B, axis=0) for z in zero_outs]
        outs = fn(*concat_in, *concat_zero)
        oi = out_names.index("out16")
        full = np.asarray(outs[oi])
        per_core_rows = out_avals[oi].shape[0]
        return full.reshape(B, per_core_rows, *out_avals[oi].shape[1:])

    _NC_CACHE["runner"] = run
    return run


def kernel(x, adj, W, b):
    in_maps = make_in_maps(x, adj, W, b)
    try:
        run = _get_runner()
        raw = run(in_maps)
    except Exception:
        from concourse.bass_utils import run_bass_kernel_spmd

        res = run_bass_kernel_spmd(_get_nc(), in_maps, list(range(B)))
        raw = np.stack([res.results[i]["out16"] for i in range(B)], axis=0)
    return _finish_host(raw, b)
